# revision 1
# baseline (speedup 1.0000x reference)
"""LayerNorm-LSTM (ContainedLSTM) Trainium2 kernel.

Strategy: data-parallel over batch B=128 -> 16 rows per core x 8 cores.
Recurrence over S=1024 runs locally per core; no collectives.

Per core layout:
  - h, c, out state: [16, 256] f32, B on partitions (LayerNorm reduces on
    the free dim via bn_stats/bn_aggr).
  - Gates: one PSUM tile [16, 1024] accumulating 4 K=128 matmuls
    (2 chunks of hn^T @ W_hh^T + 2 chunks of x_t^T @ W_ih^T), all bf16
    operands, f32 accumulation.
  - x is pre-transposed on host to [I, S, B] bf16 and resides wholly in
    SBUF (64KB/partition), so the step loop does no DMA at all.
  - Mask folded on host into a one-hot "write trigger" w[b,t]: out += h_t*w_t.
"""

import numpy as np
import ml_dtypes

import concourse.bass as bass
import concourse.bacc as bacc
import concourse.tile as tile
from concourse import mybir
from concourse.masks import make_identity
from concourse.bass_utils import run_bass_kernel_spmd

F32 = mybir.dt.float32
BF16 = mybir.dt.bfloat16

I = 256
H = 256
B = 128
S = 1024
EPS = 1e-5
NCORES = 8
BL = B // NCORES  # 16 batch rows per core

SCH = 16   # dynamic-loop chunk count
SIN = S // SCH  # 64 steps per chunk (static inner)


def _build_bass():
    nc = bacc.Bacc(
        "TRN2", target_bir_lowering=False, debug=False, num_devices=NCORES
    )

    xT_d = nc.dram_tensor("xT", [2, 128, S, BL], BF16, kind="ExternalInput")
    wcat_d = nc.dram_tensor("wcat", [4, 128, 4 * H], BF16, kind="ExternalInput")
    bias_d = nc.dram_tensor("biasb", [4 * H], F32, kind="ExternalInput")
    gamma_d = nc.dram_tensor("gamma", [H], F32, kind="ExternalInput")
    beta_d = nc.dram_tensor("beta", [H], F32, kind="ExternalInput")
    w_d = nc.dram_tensor("wtrig", [BL, S], F32, kind="ExternalInput")
    out_d = nc.dram_tensor("out", [BL, H], F32, kind="ExternalOutput")

    AF = mybir.ActivationFunctionType
    OP = mybir.AluOpType

    with tile.TileContext(nc) as tc:
        with (
            tc.tile_pool(name="consts", bufs=1) as consts,
            tc.tile_pool(name="state", bufs=1) as state,
            tc.tile_pool(name="sp", bufs=3) as sp,
            tc.tile_pool(name="gpool", bufs=2, space="PSUM") as gpool,
            tc.tile_pool(name="tpool", bufs=4, space="PSUM") as tpool,
        ):
            x_sb = consts.tile([128, 2, SCH, SIN, BL], BF16)
            wc_sb = consts.tile([128, 4, 4 * H], BF16)
            bias_sb = consts.tile([BL, 4 * H], F32)
            gamma_sb = consts.tile([BL, H], F32)
            beta_sb = consts.tile([BL, H], F32)
            w_sb = consts.tile([BL, SCH, SIN], F32)
            eps_sb = consts.tile([BL, 1], F32)
            ident = consts.tile([128, 128], BF16)

            h = state.tile([BL, H], F32)
            c = state.tile([BL, H], F32)
            outacc = state.tile([BL, H], F32)
            hnT = state.tile([128, 2, BL], BF16)

            # --- loads / init ---
            xr = x_sb.rearrange("p k a j b -> p k (a j) b")
            for k in range(2):
                nc.sync.dma_start(out=xr[:, k, :, :], in_=xT_d[k])
            for k in range(4):
                nc.gpsimd.dma_start(out=wc_sb[:, k, :], in_=wcat_d[k])
            nc.gpsimd.dma_start(
                out=bias_sb, in_=bias_d[None, :].to_broadcast([BL, 4 * H])
            )
            nc.gpsimd.dma_start(
                out=gamma_sb, in_=gamma_d[None, :].to_broadcast([BL, H])
            )
            nc.gpsimd.dma_start(
                out=beta_sb, in_=beta_d[None, :].to_broadcast([BL, H])
            )
            nc.gpsimd.dma_start(
                out=w_sb.rearrange("b a j -> b (a j)"), in_=w_d[:, :]
            )
            nc.vector.memset(eps_sb, EPS)
            make_identity(nc, ident)
            nc.vector.memset(h, 0.0)
            nc.vector.memset(c, 0.0)
            nc.vector.memset(outacc, 0.0)

            xflat = x_sb.rearrange("p k a j b -> p k (a j) b")
            wflat = w_sb.rearrange("b a j -> b (a j)")
            for t in range(S):
                    # LayerNorm(h) -> hn (f32) -> bf16 transposed hnT
                    stats = sp.tile([BL, 6], F32)
                    nc.vector.bn_stats(out=stats, in_=h)
                    mv = sp.tile([BL, 2], F32)
                    nc.vector.bn_aggr(out=mv, in_=stats)
                    std = sp.tile([BL, 1], F32)
                    nc.scalar.activation(
                        out=std, in_=mv[:, 1:2], func=AF.Sqrt, bias=eps_sb
                    )
                    rstd = sp.tile([BL, 1], F32)
                    nc.vector.reciprocal(out=rstd, in_=std)
                    hn = sp.tile([BL, H], F32)
                    nc.vector.tensor_scalar(
                        out=hn, in0=h, scalar1=mv[:, 0:1], scalar2=rstd,
                        op0=OP.subtract, op1=OP.mult,
                    )
                    hng = sp.tile([BL, H], F32)
                    nc.vector.tensor_mul(out=hng, in0=hn, in1=gamma_sb)
                    hnb = sp.tile([BL, H], BF16)
                    nc.vector.tensor_add(out=hnb, in0=hng, in1=beta_sb)
                    for k in range(2):
                        tp = tpool.tile([128, BL], BF16, tag="tp")
                        nc.tensor.transpose(
                            tp, hnb[:, k * 128:(k + 1) * 128], ident[:BL, :BL]
                        )
                        nc.vector.tensor_copy(out=hnT[:, k, :], in_=tp)

                    # gates = [hn; x_t] @ [W_hh^T; W_ih^T] + b  (PSUM f32)
                    gp = gpool.tile([BL, 4 * H], F32)
                    for nh in range(2):
                        ns = slice(nh * 512, (nh + 1) * 512)
                        nc.tensor.matmul(
                            gp[:, ns], hnT[:, 0, :], wc_sb[:, 0, ns],
                            start=True, stop=False,
                        )
                        nc.tensor.matmul(
                            gp[:, ns], hnT[:, 1, :], wc_sb[:, 1, ns],
                            start=False, stop=False,
                        )
                        nc.tensor.matmul(
                            gp[:, ns], xflat[:, 0, t, :], wc_sb[:, 2, ns],
                            start=False, stop=False,
                        )
                        nc.tensor.matmul(
                            gp[:, ns], xflat[:, 1, t, :], wc_sb[:, 3, ns],
                            start=False, stop=True,
                        )
                    gb = sp.tile([BL, 4 * H], F32)
                    nc.vector.tensor_add(out=gb, in0=gp, in1=bias_sb)

                    si = sp.tile([BL, H], F32)
                    nc.scalar.activation(out=si, in_=gb[:, 0:256], func=AF.Sigmoid)
                    sf = sp.tile([BL, H], F32)
                    nc.scalar.activation(out=sf, in_=gb[:, 256:512], func=AF.Sigmoid)
                    tg = sp.tile([BL, H], F32)
                    nc.scalar.activation(out=tg, in_=gb[:, 512:768], func=AF.Tanh)
                    so = sp.tile([BL, H], F32)
                    nc.scalar.activation(out=so, in_=gb[:, 768:1024], func=AF.Sigmoid)

                    t1 = sp.tile([BL, H], F32)
                    nc.vector.tensor_mul(out=t1, in0=sf, in1=c)
                    t2 = sp.tile([BL, H], F32)
                    nc.vector.tensor_mul(out=t2, in0=si, in1=tg)
                    nc.vector.tensor_add(out=c, in0=t1, in1=t2)
                    tcn = sp.tile([BL, H], F32)
                    nc.scalar.activation(out=tcn, in_=c, func=AF.Tanh)
                    nc.vector.tensor_mul(out=h, in0=so, in1=tcn)

                    hw = sp.tile([BL, H], F32)
                    nc.vector.tensor_scalar_mul(
                        out=hw, in0=h, scalar1=wflat[:, t:t + 1]
                    )
                    nc.vector.tensor_add(out=outacc, in0=outacc, in1=hw)

            nc.sync.dma_start(out=out_d[:, :], in_=outacc)

    nc.compile()
    return nc


_NC_CACHE = {}


def _get_nc():
    if "nc" not in _NC_CACHE:
        _NC_CACHE["nc"] = _build_bass()
    return _NC_CACHE["nc"]


def kernel(x, mask, W_ih, W_hh, b_ih, b_hh, gamma, beta):
    x = np.asarray(x, dtype=np.float32)
    mask = np.asarray(mask)
    W_ih = np.asarray(W_ih, dtype=np.float32)
    W_hh = np.asarray(W_hh, dtype=np.float32)
    b_ih = np.asarray(b_ih, dtype=np.float32)
    b_hh = np.asarray(b_hh, dtype=np.float32)
    gamma = np.asarray(gamma, dtype=np.float32)
    beta = np.asarray(beta, dtype=np.float32)

    # shared params
    whhT = np.ascontiguousarray(W_hh.T)  # [H, 4H]
    wihT = np.ascontiguousarray(W_ih.T)  # [I, 4H]
    wcat = np.stack(
        [whhT[0:128], whhT[128:256], wihT[0:128], wihT[128:256]]
    ).astype(ml_dtypes.bfloat16)  # [4,128,1024]
    biasb = (b_ih + b_hh).astype(np.float32)

    in_maps = []
    for cidx in range(NCORES):
        sl = slice(cidx * BL, (cidx + 1) * BL)
        xc = x[sl]  # [16, S, I]
        xT = np.ascontiguousarray(xc.transpose(2, 1, 0))  # [I, S, B]
        xT = xT.reshape(2, 128, S, BL).astype(ml_dtypes.bfloat16)

        m = mask[sl]  # [16, S] bool; True => keep old out
        w = np.zeros((BL, S), dtype=np.float32)
        for b in range(BL):
            idx = np.nonzero(~m[b])[0]
            if len(idx):
                w[b, idx[-1]] = 1.0

        in_maps.append({
            "xT": xT,
            "wcat": wcat,
            "biasb": biasb,
            "gamma": gamma,
            "beta": beta,
            "wtrig": w,
        })

    nc = _get_nc()
    import os
    trace = os.environ.get("KTRACE", "0") == "1"
    res = run_bass_kernel_spmd(
        nc, in_maps, core_ids=list(range(NCORES)), trace=trace
    )
    if trace:
        print(f"HW exec time: {res.exec_time_ns} ns")
        print(f"trace: {res.instructions_and_trace}"[:300])
    out = np.concatenate([r["out"] for r in res.results], axis=0)
    return out.astype(np.float32)


if __name__ == "__main__":
    nc = _build_bass()
    print("built ok")



# revision 5
# speedup vs baseline: 1.8342x; 1.8342x over previous
"""LayerNorm-LSTM (ContainedLSTM) Trainium2 kernel, v2.

Strategy: data-parallel over batch B=128 -> 16 rows per core x 8 cores.
Recurrence over S=1024 runs locally per core; no collectives.

v2 changes vs v1 (14.55 ms):
  - gamma/beta folded into W_hh' and bias' on host (removes 2 TT ops/step).
  - bias added via K=1 ones-matmul PSUM accumulation (removes [16,1024]
    TT add; activations read PSUM directly).
  - rstd = (var+eps)^-1/2 computed on the vector engine via bitcast
    Newton iteration -> no Sqrt on ScalarE -> zero ACT_TABLE_LOADs in
    the loop (saves ~2.6 us/step of table swaps).
  - gates reordered [i,f,o,g] on host: one sigmoid over [16,512] (i,f),
    tanh over g, sigmoid over o -> 3 ACTIVATEs instead of 4 + better
    chaining.
  - x-part matmuls + bias matmul accumulate into double-buffered PSUM
    before h_t is ready (off the critical path).
  - out-accumulation fused into one scalar_tensor_tensor placed in a
    vector-idle slot.
  - bf16 h/hn state for cheaper DVE ops.
"""

import numpy as np
import ml_dtypes

import concourse.bass as bass
import concourse.bacc as bacc
import concourse.tile as tile
from concourse import mybir
from concourse.masks import make_identity
from concourse.bass_utils import run_bass_kernel_spmd

F32 = mybir.dt.float32
I32 = mybir.dt.int32
BF16 = mybir.dt.bfloat16

I = 256
H = 256
B = 128
S = 1024
EPS = 1e-5
NCORES = 8
BL = B // NCORES  # 16 batch rows per core
MAGIC = 0x5F3759DF


def _build_bass():
    nc = bacc.Bacc(
        "TRN2", target_bir_lowering=False, debug=False, num_devices=NCORES
    )

    xT_d = nc.dram_tensor("xT", [2, 128, S, BL], BF16, kind="ExternalInput")
    wcat_d = nc.dram_tensor("wcat", [4, 128, 4 * H], BF16, kind="ExternalInput")
    biasrow_d = nc.dram_tensor("biasrow", [1, 4 * H], BF16, kind="ExternalInput")
    w_d = nc.dram_tensor("wtrig", [BL, S], F32, kind="ExternalInput")
    out_d = nc.dram_tensor("out", [BL, H], F32, kind="ExternalOutput")

    AF = mybir.ActivationFunctionType
    OP = mybir.AluOpType

    with tile.TileContext(nc) as tc:
        with (
            tc.tile_pool(name="consts", bufs=1) as consts,
            tc.tile_pool(name="state", bufs=1) as state,
            tc.tile_pool(name="sp", bufs=3) as sp,
            tc.tile_pool(name="gpool", bufs=2, space="PSUM") as gpool,
            tc.tile_pool(name="tpool", bufs=2, space="PSUM") as tpool,
            tc.tile_pool(name="cpool", bufs=1, space="PSUM") as cpool,
        ):
            x_sb = consts.tile([128, 2, S, BL], BF16)
            wc_sb = consts.tile([128, 4, 4 * H], BF16)
            brow_sb = consts.tile([1, 4 * H], BF16)
            ones_sb = consts.tile([1, BL], BF16)
            w_sb = consts.tile([BL, S], F32)
            ident = consts.tile([128, 128], BF16)

            h = state.tile([BL, H], BF16)
            c = cpool.tile([BL, H], F32)  # PSUM resident cell state
            outacc = state.tile([BL, H], F32)
            hnT = state.tile([128, 2 * BL], BF16)

            # --- loads / init ---
            for k in range(2):
                nc.sync.dma_start(out=x_sb[:, k, :, :], in_=xT_d[k])
            for k in range(4):
                nc.gpsimd.dma_start(out=wc_sb[:, k, :], in_=wcat_d[k])
            nc.gpsimd.dma_start(out=brow_sb, in_=biasrow_d[:, :])
            nc.gpsimd.dma_start(out=w_sb, in_=w_d[:, :])
            nc.vector.memset(ones_sb, 1.0)
            make_identity(nc, ident)
            nc.vector.memset(h, 0.0)
            nc.vector.memset(c, 0.0)
            nc.vector.memset(outacc, 0.0)

            def accum_xb(gp, t):
                """Bias + x-part accumulation into gates PSUM (no h dep)."""
                for nh in range(2):
                    ns = slice(nh * 512, (nh + 1) * 512)
                    nc.tensor.matmul(
                        gp[:, ns], ones_sb, brow_sb[:, ns],
                        start=True, stop=False,
                    )
                    nc.tensor.matmul(
                        gp[:, ns], x_sb[:, 0, t, :], wc_sb[:, 2, ns],
                        start=False, stop=False,
                    )
                    nc.tensor.matmul(
                        gp[:, ns], x_sb[:, 1, t, :], wc_sb[:, 3, ns],
                        start=False, stop=False,
                    )

            # prime step-0 x/bias accumulation
            gp_cur = gpool.tile([BL, 4 * H], F32, name="gp", tag="gp")
            accum_xb(gp_cur, 0)

            for t in range(S):
                # ---- LayerNorm stats ----
                stats = sp.tile([BL, 6], F32)
                nc.vector.bn_stats(out=stats, in_=h)
                mv = sp.tile([BL, 2], F32)
                nc.vector.bn_aggr(out=mv, in_=stats)

                # ---- rstd = (var+eps)^-1/2 via bitcast Newton (DVE) ----
                vpe = sp.tile([BL, 1], F32)
                nc.vector.tensor_scalar(
                    out=vpe, in0=mv[:, 1:2], scalar1=EPS, scalar2=None,
                    op0=OP.add,
                )
                ti = sp.tile([BL, 1], I32)
                nc.vector.tensor_scalar(
                    out=ti, in0=vpe.bitcast(I32), scalar1=1, scalar2=None,
                    op0=OP.logical_shift_right,
                )
                y0i = sp.tile([BL, 1], I32)
                nc.vector.tensor_scalar(
                    out=y0i, in0=ti, scalar1=MAGIC, scalar2=-1,
                    op0=OP.subtract, op1=OP.mult,
                )
                y0 = y0i.bitcast(F32)
                h1 = sp.tile([BL, 1], F32)
                nc.vector.scalar_tensor_tensor(
                    out=h1, in0=y0, scalar=-0.5, in1=y0,
                    op0=OP.mult, op1=OP.mult,
                )  # -0.5*y0^2
                h2 = sp.tile([BL, 1], F32)
                nc.vector.tensor_mul(out=h2, in0=h1, in1=vpe)  # -0.5*v*y0^2
                rstd = sp.tile([BL, 1], F32)
                nc.vector.scalar_tensor_tensor(
                    out=rstd, in0=h2, scalar=1.5, in1=y0,
                    op0=OP.add, op1=OP.mult,
                )  # (1.5 - 0.5*v*y0^2) * y0

                # ---- hn = (h - mu) * rstd (bf16) ----
                hnb = sp.tile([BL, H], BF16)
                nc.vector.tensor_scalar(
                    out=hnb, in0=h, scalar1=mv[:, 0:1], scalar2=rstd,
                    op0=OP.subtract, op1=OP.mult,
                )

                # ---- transpose hn -> hnT [128, 32] ----
                tp = tpool.tile([128, 2 * BL], BF16, tag="tp")
                for k in range(2):
                    nc.tensor.transpose(
                        tp[:, k * BL:(k + 1) * BL],
                        hnb[:, k * 128:(k + 1) * 128],
                        ident[:BL, :BL],
                    )
                nc.vector.tensor_copy(out=hnT, in_=tp)

                # ---- out-accumulation for step t-1's h is fused below; here
                # accumulate current outacc while PE/scalar work (h of step t
                # is not yet overwritten: we use w for this step after h is
                # updated at the end; so do nothing here).

                # ---- hn matmuls (critical) ----
                gp = gp_cur
                for nh in range(2):
                    ns = slice(nh * 512, (nh + 1) * 512)
                    nc.tensor.matmul(
                        gp[:, ns], hnT[:, 0:BL], wc_sb[:, 0, ns],
                        start=False, stop=False,
                    )
                    nc.tensor.matmul(
                        gp[:, ns], hnT[:, BL:2 * BL], wc_sb[:, 1, ns],
                        start=False, stop=True,
                    )

                # ---- next step x/bias accumulation (off-path) ----
                if t + 1 < S:
                    gp_next = gpool.tile([BL, 4 * H], F32, name="gp", tag="gp")
                    accum_xb(gp_next, t + 1)
                else:
                    gp_next = None

                # ---- activations: [i, f, o, g] layout ----
                sif = sp.tile([BL, 2 * H], BF16)
                nc.scalar.activation(out=sif, in_=gp[:, 0:512], func=AF.Sigmoid)
                tg = sp.tile([BL, H], BF16)
                nc.scalar.activation(out=tg, in_=gp[:, 768:1024], func=AF.Tanh)
                so = sp.tile([BL, H], BF16)
                nc.scalar.activation(out=so, in_=gp[:, 512:768], func=AF.Sigmoid)

                # ---- cell update ----
                t1 = sp.tile([BL, H], F32)
                nc.vector.tensor_mul(out=t1, in0=sif[:, 256:512], in1=c)
                t2 = sp.tile([BL, H], BF16)
                nc.vector.tensor_mul(out=t2, in0=sif[:, 0:256], in1=tg)
                nc.vector.tensor_add(out=c, in0=t1, in1=t2)
                tcn = sp.tile([BL, H], BF16)
                nc.scalar.activation(out=tcn, in_=c, func=AF.Tanh)

                # outacc += h_t * w_t happens after h is written; to keep the
                # vector queue free at the step head, fuse into one op here
                # (h still holds step t's value only after the mul below).
                nc.vector.tensor_mul(out=h, in0=so, in1=tcn)
                nc.vector.scalar_tensor_tensor(
                    out=outacc, in0=h, scalar=w_sb[:, t:t + 1], in1=outacc,
                    op0=OP.mult, op1=OP.add,
                )
                gp_cur = gp_next

            nc.sync.dma_start(out=out_d[:, :], in_=outacc)

    nc.compile()
    return nc


_NC_CACHE = {}


def _get_nc():
    if "nc" not in _NC_CACHE:
        _NC_CACHE["nc"] = _build_bass()
    return _NC_CACHE["nc"]


def kernel(x, mask, W_ih, W_hh, b_ih, b_hh, gamma, beta):
    x = np.asarray(x, dtype=np.float32)
    mask = np.asarray(mask)
    W_ih = np.asarray(W_ih, dtype=np.float32)
    W_hh = np.asarray(W_hh, dtype=np.float32)
    b_ih = np.asarray(b_ih, dtype=np.float32)
    b_hh = np.asarray(b_hh, dtype=np.float32)
    gamma = np.asarray(gamma, dtype=np.float32)
    beta = np.asarray(beta, dtype=np.float32)

    # Fold gamma/beta into W_hh and bias:
    #   gates = ((h-mu)*rstd*gamma + beta) @ W_hh.T + x @ W_ih.T + b
    #         = ((h-mu)*rstd) @ (gamma[:,None]*W_hh.T) + x @ W_ih.T + b'
    #   b' = b_ih + b_hh + beta @ W_hh.T
    whhT = gamma[:, None] * W_hh.T  # [H, 4H]
    wihT = np.ascontiguousarray(W_ih.T)  # [I, 4H]
    biasb = b_ih + b_hh + beta @ W_hh.T  # [4H]

    # Reorder gates [i, f, g, o] -> [i, f, o, g] so sigmoid gates are
    # contiguous in [0:768] and tanh in [768:1024].
    perm = np.concatenate([
        np.arange(0, 2 * H),          # i, f
        np.arange(3 * H, 4 * H),      # o
        np.arange(2 * H, 3 * H),      # g
    ])
    whhT = np.ascontiguousarray(whhT[:, perm])
    wihT = np.ascontiguousarray(wihT[:, perm])
    biasb = np.ascontiguousarray(biasb[perm])

    wcat = np.stack(
        [whhT[0:128], whhT[128:256], wihT[0:128], wihT[128:256]]
    ).astype(ml_dtypes.bfloat16)  # [4,128,1024]
    biasrow = biasb[None, :].astype(ml_dtypes.bfloat16)  # [1, 1024]

    in_maps = []
    for cidx in range(NCORES):
        sl = slice(cidx * BL, (cidx + 1) * BL)
        xc = x[sl]  # [16, S, I]
        xT = np.ascontiguousarray(xc.transpose(2, 1, 0))  # [I, S, BL]
        xT = xT.reshape(2, 128, S, BL).astype(ml_dtypes.bfloat16)

        m = mask[sl]  # [16, S] bool; True => keep old out
        w = np.zeros((BL, S), dtype=np.float32)
        for b in range(BL):
            idx = np.nonzero(~m[b])[0]
            if len(idx):
                w[b, idx[-1]] = 1.0

        in_maps.append({
            "xT": xT,
            "wcat": wcat,
            "biasrow": biasrow,
            "wtrig": w,
        })

    nc = _get_nc()
    import os
    trace = os.environ.get("KTRACE", "0") == "1"
    res = run_bass_kernel_spmd(
        nc, in_maps, core_ids=list(range(NCORES)), trace=trace
    )
    if trace:
        print(f"HW exec time: {res.exec_time_ns} ns")
    out = np.concatenate([r["out"] for r in res.results], axis=0)
    return out.astype(np.float32)


if __name__ == "__main__":
    nc = _build_bass()
    print("built ok")
